# revision 9
# baseline (speedup 1.0000x reference)
"""CoverageLoss kernel for 8 Trainium2 NeuronCores.

Loss = size(latent_states) + size(latent_actions)
     + cov(state_samples, latent_states) + cov(action_samples, latent_actions)
cov(S, L): d = cdist_l1(S, L); sm4 = 4 smallest per row; tail = sm4.mean(-1);
           far = top64(tail); loss = mean(sm4[far] ** 2)

Device strategy (4 latent shards x 2 sample shards; per core 1024 samples x
2048 latents per coverage term):
  - L2 screening score via TensorE: score[s, l] = <s, l> - |l|^2 / 2
    (f16 operands, K=128 with a ones-row on the sample side multiplying the
    -|l|^2/2 row on the latent side; fp32 PSUM accumulate).  Ranking latents
    by score == ranking by L2 distance; the true L1-nearest neighbors are
    recovered by an exact host-side refine over a generous candidate set.
  - Tournament reduce on device: ACT copies the second half of each
    [128, 2048] PSUM score tile to SBUF; DVE max-reduces halves twice down to
    [128, 512] f16 ("quad" representatives, quad j = cols {j, j+512, j+1024,
    j+1536}).  Only the reduced arrays ship to the host: top-k selection over
    them plus exact refine of the expanded members is O(samples * 256) work.
  - Size losses: per-row relu(|x|_1 - 1)^2 on device; host means.
Host: per (sample, shard) take top-16 quads, expand x4 -> 64 candidates,
  exact L1 distances, merge 4 shards (256 candidates), sm4 / tails / top-64
  far samples / final scalar in float64.

Validated against a float sim: rel err ~5e-4 (gate is 2e-2); the candidate
margin makes the result insensitive to f16 rounding of the screening scores.
"""

from contextlib import ExitStack

import numpy as np

import concourse.bass as bass
import concourse.bacc as bacc
import concourse.mybir as mybir
import concourse.tile as tile
from concourse.bass_utils import run_bass_kernel_spmd

NLAT, ES, EA = 8192, 64, 32
NSMP = 2048
A_SHARDS, B_SHARDS = 4, 2
NL = NLAT // A_SHARDS              # 2048 latents per core
MS = NSMP // B_SHARDS              # 1024 samples per core
NTILES = MS // 128                 # 8 sample tiles per coverage term
NCHUNK = NL // 512                 # 4 psum column chunks
RED = NL // 4                      # 512 quad representatives
TOPK = 16                          # host: quads refined per (sample, shard)
TAIL, FAR = 4, 64

F32 = mybir.dt.float32
F16 = mybir.dt.float16


def _cov_kernel(tc, psum, work, latT, smpT, red_out, tag):
    """Emit one coverage screening pipeline (states or actions).

    latT: [128, NL] f16 SBUF: coords rows then -|l|^2/2 row then zero pad
    smpT: [128, MS] f16 SBUF: coords rows then ones row then zero pad
    red_out: [MS, RED] f16 DRAM, quad-reduced screening scores
    """
    nc = tc.nc
    for m in range(NTILES):
        ps = psum.tile([128, NL], F32, tag="ps")
        for n in range(NCHUNK):
            nc.tensor.matmul(
                ps[:, n * 512:(n + 1) * 512],
                lhsT=smpT[:, m * 128:(m + 1) * 128],
                rhs=latT[:, n * 512:(n + 1) * 512],
                start=True, stop=True)
        # second half of the scores to SBUF so DVE reads PSUM only once
        h2 = work.tile([128, NL // 2], F32, tag="h2")
        nc.scalar.copy(h2[:], ps[:, NL // 2:])
        q1 = work.tile([128, NL // 2], F16, tag="q1")
        nc.vector.tensor_tensor(out=q1[:], in0=ps[:, :NL // 2], in1=h2[:],
                                op=mybir.AluOpType.max)
        q2 = work.tile([128, RED], F16, tag="q2")
        nc.vector.tensor_tensor(out=q2[:], in0=q1[:, :RED], in1=q1[:, RED:],
                                op=mybir.AluOpType.max)
        nc.sync.dma_start(red_out[m * 128:(m + 1) * 128, :], q2[:])


def _size_kernel(ctx, tc, e, lat_rows, sz_out, tag):
    """Per-row relu(|x|_1 - 1)^2 for a [NL, e] latent shard."""
    nc = tc.nc
    pool = ctx.enter_context(tc.tile_pool(name=f"sz_{tag}", bufs=1))
    nt = NL // 128
    lat_big = pool.tile([128, nt * e], F32, tag=f"latbig_{tag}")
    # partition-major row blocks: contiguous 4KB per partition (fast DMA);
    # row order is irrelevant because the host only takes the mean
    nc.sync.dma_start(
        lat_big[:], lat_rows.rearrange("(p m) e -> p m e", p=128))
    norms = pool.tile([128, nt], F32, tag=f"norms_{tag}")
    nc.vector.tensor_reduce(
        out=norms[:], in_=lat_big[:].rearrange("p (m e) -> p m e", m=nt),
        axis=mybir.AxisListType.X, op=mybir.AluOpType.add,
        apply_absolute_value=True)
    rl = pool.tile([128, nt], F32, tag=f"rl_{tag}")
    nc.vector.tensor_scalar(out=rl[:], in0=norms[:], scalar1=1.0, scalar2=0.0,
                            op0=mybir.AluOpType.subtract,
                            op1=mybir.AluOpType.max)
    sq = pool.tile([128, nt], F32, tag=f"sq_{tag}")
    nc.vector.tensor_tensor(out=sq[:], in0=rl[:], in1=rl[:],
                            op=mybir.AluOpType.mult)
    nc.sync.dma_start(sz_out[:, :], sq[:])


def _build_nc():
    nc = bacc.Bacc("TRN2", target_bir_lowering=False, debug=False,
                   num_devices=8)
    inp = {}
    for name, shape, dt in [
        ("latT_s", [128, NL], F16), ("latT_a", [128, NL], F16),
        ("smpT_s", [128, MS], F16), ("smpT_a", [128, MS], F16),
        ("lat_s", [NL, ES], F32), ("lat_a", [NL, EA], F32),
    ]:
        inp[name] = nc.dram_tensor(name, shape, dt, kind="ExternalInput").ap()
    out = {}
    for name, shape, dt in [
        ("red_s", [MS, RED], F16), ("red_a", [MS, RED], F16),
        ("szrows_s", [128, NL // 128], F32),
        ("szrows_a", [128, NL // 128], F32),
    ]:
        out[name] = nc.dram_tensor(name, shape, dt, kind="ExternalOutput").ap()

    with tile.TileContext(nc) as tc:
        with ExitStack() as ctx:
            big = ctx.enter_context(tc.tile_pool(name="bigin", bufs=1))
            tiles = {}
            # spread input loads across the three DMA-issuing engines so the
            # first pipeline's operands land in parallel instead of queueing
            dma_eng = {"latT_s": nc.sync, "smpT_s": nc.scalar,
                       "latT_a": nc.gpsimd, "smpT_a": nc.sync}
            for name in ("latT_s", "smpT_s", "latT_a", "smpT_a"):
                t = big.tile(list(inp[name].shape), F16, tag=name)
                dma_eng[name].dma_start(t[:], inp[name][:, :])
                tiles[name] = t

            psum = ctx.enter_context(tc.tile_pool(name="psum", bufs=2,
                                                  space="PSUM"))
            work = ctx.enter_context(tc.tile_pool(name="work", bufs=3))

            # PE warm-up: dummy matmuls with no DMA dependency run during the
            # input-load window so the HAM clock gate reaches 2.4 GHz before
            # the real matmuls start (and they then keep it warm)
            scratch = work.tile([128, 512], F16, tag="warm")
            nc.vector.memset(scratch[:], 0.0)
            ps_w = psum.tile([128, NL], F32, tag="ps")
            for i in range(12):
                nc.tensor.matmul(
                    ps_w[:, (i % 4) * 512:(i % 4 + 1) * 512],
                    lhsT=scratch[:, :128], rhs=scratch[:],
                    start=True, stop=True)

            _cov_kernel(tc, psum, work, tiles["latT_s"][:], tiles["smpT_s"][:],
                        out["red_s"], "s")
            _size_kernel(ctx, tc, ES, inp["lat_s"], out["szrows_s"], "s")
            _size_kernel(ctx, tc, EA, inp["lat_a"], out["szrows_a"], "a")
            _cov_kernel(tc, psum, work, tiles["latT_a"][:], tiles["smpT_a"][:],
                        out["red_a"], "a")
    nc.compile()
    return nc


_NC_CACHE = {}


def _get_nc():
    if "nc" not in _NC_CACHE:
        _NC_CACHE["nc"] = _build_nc()
    return _NC_CACHE["nc"]


def _aug_latT(lat):
    """[NL, e] f32 -> [128, NL] f16: coords.T, then -|l|^2/2 row, zero pad."""
    e = lat.shape[1]
    m = np.zeros((128, lat.shape[0]), np.float16)
    m[:e] = lat.T.astype(np.float16)
    m[e] = (-(lat.astype(np.float32) ** 2).sum(1) / 2).astype(np.float16)
    return m


def _aug_smpT(smp):
    """[MS, e] f32 -> [128, MS] f16: coords.T, then ones row, zero pad."""
    e = smp.shape[1]
    m = np.zeros((128, smp.shape[0]), np.float16)
    m[:e] = smp.T.astype(np.float16)
    m[e] = 1.0
    return m


def _make_in_maps(latent_states, latent_actions, state_space_samples,
                  action_space_samples):
    in_maps = []
    for core in range(8):
        a, b = core % A_SHARDS, core // A_SHARDS
        lat_s = np.ascontiguousarray(latent_states[a * NL:(a + 1) * NL],
                                     dtype=np.float32)
        lat_a = np.ascontiguousarray(latent_actions[a * NL:(a + 1) * NL],
                                     dtype=np.float32)
        smp_s = state_space_samples[b * MS:(b + 1) * MS]
        smp_a = action_space_samples[b * MS:(b + 1) * MS]
        in_maps.append({
            "latT_s": _aug_latT(lat_s), "latT_a": _aug_latT(lat_a),
            "smpT_s": _aug_smpT(smp_s), "smpT_a": _aug_smpT(smp_a),
            "lat_s": lat_s, "lat_a": lat_a,
        })
    return in_maps


def _host_cov(results, key, samples, latents):
    """Exact host refine: top-TOPK quads per (sample, shard) -> expand x4 ->
    exact L1 -> merge shards -> sm4/tails/far -> mean(sm4^2)."""
    samples = samples.astype(np.float32)
    dist = np.empty((NSMP, A_SHARDS * TOPK * 4), np.float32)
    for a in range(A_SHARDS):
        lat = latents[a * NL:(a + 1) * NL].astype(np.float32)
        red = np.concatenate(
            [results[b * A_SHARDS + a][key] for b in range(B_SHARDS)], 0
        ).astype(np.float32)                                    # [NSMP, RED]
        idx = np.argpartition(-red, TOPK, axis=1)[:, :TOPK]     # [NSMP, TOPK]
        members = (idx[:, :, None] + np.arange(0, NL, RED)[None, None, :]
                   ).reshape(NSMP, -1)                          # [NSMP, 4*TOPK]
        for lo in range(0, NSMP, 256):
            hi = lo + 256
            g = lat[members[lo:hi]]                             # [256, 64, e]
            dist[lo:hi, a * TOPK * 4:(a + 1) * TOPK * 4] = np.abs(
                samples[lo:hi, None, :] - g).sum(-1)
    dist.sort(axis=1)
    sm4 = dist[:, :TAIL].astype(np.float64)
    tails = sm4.mean(1)
    far = np.argsort(-tails)[:FAR]
    return (sm4[far] ** 2).mean()


def _host_combine(results, latent_states, latent_actions,
                  state_space_samples, action_space_samples):
    total = np.float64(0)
    sz_s = [results[a]["szrows_s"] for a in range(A_SHARDS)]
    sz_a = [results[A_SHARDS + a]["szrows_a"] for a in range(A_SHARDS)]
    total += np.concatenate([s.ravel() for s in sz_s]).mean(dtype=np.float64)
    total += np.concatenate([s.ravel() for s in sz_a]).mean(dtype=np.float64)
    total += _host_cov(results, "red_s", state_space_samples, latent_states)
    total += _host_cov(results, "red_a", action_space_samples, latent_actions)
    return np.float32(total)


def kernel(latent_states, latent_actions, state_space_samples,
           action_space_samples, _want_results=False, _trace=False):
    latent_states = np.asarray(latent_states)
    latent_actions = np.asarray(latent_actions)
    state_space_samples = np.asarray(state_space_samples)
    action_space_samples = np.asarray(action_space_samples)
    nc = _get_nc()
    in_maps = _make_in_maps(latent_states, latent_actions,
                            state_space_samples, action_space_samples)
    res = run_bass_kernel_spmd(nc, in_maps, core_ids=list(range(8)),
                               trace=_trace)
    out = _host_combine(res.results, latent_states, latent_actions,
                        state_space_samples, action_space_samples)
    if _want_results:
        return out, res
    return out


# revision 10
# speedup vs baseline: 1.0814x; 1.0814x over previous
"""CoverageLoss kernel for 8 Trainium2 NeuronCores.

Loss = size(latent_states) + size(latent_actions)
     + cov(state_samples, latent_states) + cov(action_samples, latent_actions)
cov(S, L): d = cdist_l1(S, L); sm4 = 4 smallest per row; tail = sm4.mean(-1);
           far = top64(tail); loss = mean(sm4[far] ** 2)

Device strategy (4 latent shards x 2 sample shards; per core 1024 samples x
2048 latents per coverage term):
  - L2 screening score via TensorE: score[s, l] = <s, l> - |l|^2 / 2
    (f16 operands, K=128 with a ones-row on the sample side multiplying the
    -|l|^2/2 row on the latent side; fp32 PSUM accumulate).  Ranking latents
    by score == ranking by L2 distance; the true L1-nearest neighbors are
    recovered by an exact host-side refine over a generous candidate set.
  - Tournament reduce on device: ACT copies the second half of each
    [128, 2048] PSUM score tile to SBUF; DVE max-reduces halves twice down to
    [128, 512] f16 ("quad" representatives, quad j = cols {j, j+512, j+1024,
    j+1536}).  Only the reduced arrays ship to the host: top-k selection over
    them plus exact refine of the expanded members is O(samples * 256) work.
  - Size losses: per-row relu(|x|_1 - 1)^2 on device; host means.
Host: per (sample, shard) take top-16 quads, expand x4 -> 64 candidates,
  exact L1 distances, merge 4 shards (256 candidates), sm4 / tails / top-64
  far samples / final scalar in float64.

Validated against a float sim: rel err ~5e-4 (gate is 2e-2); the candidate
margin makes the result insensitive to f16 rounding of the screening scores.
"""

from contextlib import ExitStack

import numpy as np

import concourse.bass as bass
import concourse.bacc as bacc
import concourse.mybir as mybir
import concourse.tile as tile
from concourse.bass_utils import run_bass_kernel_spmd

NLAT, ES, EA = 8192, 64, 32
NSMP = 2048
A_SHARDS, B_SHARDS = 4, 2
NL = NLAT // A_SHARDS              # 2048 latents per core
MS = NSMP // B_SHARDS              # 1024 samples per core
NTILES = MS // 128                 # 8 sample tiles per coverage term
NCHUNK = NL // 512                 # 4 psum column chunks
RED = NL // 4                      # 512 quad representatives
TOPK = 16                          # host: quads refined per (sample, shard)
TAIL, FAR = 4, 64

F32 = mybir.dt.float32
F16 = mybir.dt.float16


def _cov_kernel(tc, psum, work, latT, smpT, red_out, tag):
    """Emit one coverage screening pipeline (states or actions).

    latT: [128, NL] f16 SBUF: coords rows then -|l|^2/2 row then zero pad
    smpT: [128, MS] f16 SBUF: coords rows then ones row then zero pad
    red_out: [MS, RED] f16 DRAM, quad-reduced screening scores
    """
    nc = tc.nc
    for m in range(NTILES):
        ps = psum.tile([128, NL], F32, tag="ps")
        for n in range(NCHUNK):
            nc.tensor.matmul(
                ps[:, n * 512:(n + 1) * 512],
                lhsT=smpT[:, m * 128:(m + 1) * 128],
                rhs=latT[:, n * 512:(n + 1) * 512],
                start=True, stop=True)
        # second half of the scores to SBUF so DVE reads PSUM only once
        h2 = work.tile([128, NL // 2], F32, tag="h2")
        nc.scalar.copy(h2[:], ps[:, NL // 2:])
        q1 = work.tile([128, NL // 2], F16, tag="q1")
        nc.vector.tensor_tensor(out=q1[:], in0=ps[:, :NL // 2], in1=h2[:],
                                op=mybir.AluOpType.max)
        q2 = work.tile([128, RED], F16, tag="q2")
        nc.vector.tensor_tensor(out=q2[:], in0=q1[:, :RED], in1=q1[:, RED:],
                                op=mybir.AluOpType.max)
        nc.sync.dma_start(red_out[m * 128:(m + 1) * 128, :], q2[:])


def _size_kernel(ctx, tc, e, lat_rows, sz_out, tag):
    """Per-row relu(|x|_1 - 1)^2 for a [NL, e] latent shard."""
    nc = tc.nc
    pool = ctx.enter_context(tc.tile_pool(name=f"sz_{tag}", bufs=1))
    nt = NL // 128
    lat_big = pool.tile([128, nt * e], F32, tag=f"latbig_{tag}")
    # partition-major row blocks: contiguous 4KB per partition (fast DMA);
    # row order is irrelevant because the host only takes the mean
    nc.sync.dma_start(
        lat_big[:], lat_rows.rearrange("(p m) e -> p m e", p=128))
    norms = pool.tile([128, nt], F32, tag=f"norms_{tag}")
    nc.vector.tensor_reduce(
        out=norms[:], in_=lat_big[:].rearrange("p (m e) -> p m e", m=nt),
        axis=mybir.AxisListType.X, op=mybir.AluOpType.add,
        apply_absolute_value=True)
    rl = pool.tile([128, nt], F32, tag=f"rl_{tag}")
    nc.vector.tensor_scalar(out=rl[:], in0=norms[:], scalar1=1.0, scalar2=0.0,
                            op0=mybir.AluOpType.subtract,
                            op1=mybir.AluOpType.max)
    sq = pool.tile([128, nt], F32, tag=f"sq_{tag}")
    nc.vector.tensor_tensor(out=sq[:], in0=rl[:], in1=rl[:],
                            op=mybir.AluOpType.mult)
    nc.sync.dma_start(sz_out[:, :], sq[:])


def _build_nc():
    nc = bacc.Bacc("TRN2", target_bir_lowering=False, debug=False,
                   num_devices=8)
    inp = {}
    for name, shape, dt in [
        ("latT_s", [128, NL], F16), ("latT_a", [128, NL], F16),
        ("smpT_s", [128, MS], F16), ("smpT_a", [128, MS], F16),
        ("lat_s", [NL, ES], F32), ("lat_a", [NL, EA], F32),
    ]:
        inp[name] = nc.dram_tensor(name, shape, dt, kind="ExternalInput").ap()
    out = {}
    for name, shape, dt in [
        ("red_s", [MS, RED], F16), ("red_a", [MS, RED], F16),
        ("szrows_s", [128, NL // 128], F32),
        ("szrows_a", [128, NL // 128], F32),
    ]:
        out[name] = nc.dram_tensor(name, shape, dt, kind="ExternalOutput").ap()

    with tile.TileContext(nc) as tc:
        with ExitStack() as ctx:
            big = ctx.enter_context(tc.tile_pool(name="bigin", bufs=1))
            tiles = {}
            # spread input loads across the three DMA-issuing engines so the
            # first pipeline's operands land in parallel instead of queueing
            dma_eng = {"latT_s": nc.sync, "smpT_s": nc.scalar,
                       "latT_a": nc.gpsimd, "smpT_a": nc.sync}
            for name in ("latT_s", "smpT_s", "latT_a", "smpT_a"):
                t = big.tile(list(inp[name].shape), F16, tag=name)
                dma_eng[name].dma_start(t[:], inp[name][:, :])
                tiles[name] = t

            psum = ctx.enter_context(tc.tile_pool(name="psum", bufs=2,
                                                  space="PSUM"))
            work = ctx.enter_context(tc.tile_pool(name="work", bufs=3))

            # PE warm-up: dummy matmuls with no DMA dependency run during the
            # input-load window so the HAM clock gate reaches 2.4 GHz before
            # the real matmuls start (and they then keep it warm)
            scratch = work.tile([128, 512], F16, tag="warm")
            nc.gpsimd.memset(scratch[:], 0.0)
            ps_w = psum.tile([128, NL], F32, tag="ps")
            for i in range(6):
                nc.tensor.matmul(
                    ps_w[:, (i % 4) * 512:(i % 4 + 1) * 512],
                    lhsT=scratch[:, :128], rhs=scratch[:],
                    start=True, stop=True)

            _cov_kernel(tc, psum, work, tiles["latT_s"][:], tiles["smpT_s"][:],
                        out["red_s"], "s")
            _cov_kernel(tc, psum, work, tiles["latT_a"][:], tiles["smpT_a"][:],
                        out["red_a"], "a")
            # size kernels last: their vector ops must not precede the
            # coverage reduces in the Vector engine's static program order
            _size_kernel(ctx, tc, ES, inp["lat_s"], out["szrows_s"], "s")
            _size_kernel(ctx, tc, EA, inp["lat_a"], out["szrows_a"], "a")
    nc.compile()
    return nc


_NC_CACHE = {}


def _get_nc():
    if "nc" not in _NC_CACHE:
        _NC_CACHE["nc"] = _build_nc()
    return _NC_CACHE["nc"]


def _aug_latT(lat):
    """[NL, e] f32 -> [128, NL] f16: coords.T, then -|l|^2/2 row, zero pad."""
    e = lat.shape[1]
    m = np.zeros((128, lat.shape[0]), np.float16)
    m[:e] = lat.T.astype(np.float16)
    m[e] = (-(lat.astype(np.float32) ** 2).sum(1) / 2).astype(np.float16)
    return m


def _aug_smpT(smp):
    """[MS, e] f32 -> [128, MS] f16: coords.T, then ones row, zero pad."""
    e = smp.shape[1]
    m = np.zeros((128, smp.shape[0]), np.float16)
    m[:e] = smp.T.astype(np.float16)
    m[e] = 1.0
    return m


def _make_in_maps(latent_states, latent_actions, state_space_samples,
                  action_space_samples):
    in_maps = []
    for core in range(8):
        a, b = core % A_SHARDS, core // A_SHARDS
        lat_s = np.ascontiguousarray(latent_states[a * NL:(a + 1) * NL],
                                     dtype=np.float32)
        lat_a = np.ascontiguousarray(latent_actions[a * NL:(a + 1) * NL],
                                     dtype=np.float32)
        smp_s = state_space_samples[b * MS:(b + 1) * MS]
        smp_a = action_space_samples[b * MS:(b + 1) * MS]
        in_maps.append({
            "latT_s": _aug_latT(lat_s), "latT_a": _aug_latT(lat_a),
            "smpT_s": _aug_smpT(smp_s), "smpT_a": _aug_smpT(smp_a),
            "lat_s": lat_s, "lat_a": lat_a,
        })
    return in_maps


def _host_cov(results, key, samples, latents):
    """Exact host refine: top-TOPK quads per (sample, shard) -> expand x4 ->
    exact L1 -> merge shards -> sm4/tails/far -> mean(sm4^2)."""
    samples = samples.astype(np.float32)
    dist = np.empty((NSMP, A_SHARDS * TOPK * 4), np.float32)
    for a in range(A_SHARDS):
        lat = latents[a * NL:(a + 1) * NL].astype(np.float32)
        red = np.concatenate(
            [results[b * A_SHARDS + a][key] for b in range(B_SHARDS)], 0
        ).astype(np.float32)                                    # [NSMP, RED]
        idx = np.argpartition(-red, TOPK, axis=1)[:, :TOPK]     # [NSMP, TOPK]
        members = (idx[:, :, None] + np.arange(0, NL, RED)[None, None, :]
                   ).reshape(NSMP, -1)                          # [NSMP, 4*TOPK]
        for lo in range(0, NSMP, 256):
            hi = lo + 256
            g = lat[members[lo:hi]]                             # [256, 64, e]
            dist[lo:hi, a * TOPK * 4:(a + 1) * TOPK * 4] = np.abs(
                samples[lo:hi, None, :] - g).sum(-1)
    dist.sort(axis=1)
    sm4 = dist[:, :TAIL].astype(np.float64)
    tails = sm4.mean(1)
    far = np.argsort(-tails)[:FAR]
    return (sm4[far] ** 2).mean()


def _host_combine(results, latent_states, latent_actions,
                  state_space_samples, action_space_samples):
    total = np.float64(0)
    sz_s = [results[a]["szrows_s"] for a in range(A_SHARDS)]
    sz_a = [results[A_SHARDS + a]["szrows_a"] for a in range(A_SHARDS)]
    total += np.concatenate([s.ravel() for s in sz_s]).mean(dtype=np.float64)
    total += np.concatenate([s.ravel() for s in sz_a]).mean(dtype=np.float64)
    total += _host_cov(results, "red_s", state_space_samples, latent_states)
    total += _host_cov(results, "red_a", action_space_samples, latent_actions)
    return np.float32(total)


def kernel(latent_states, latent_actions, state_space_samples,
           action_space_samples, _want_results=False, _trace=False):
    latent_states = np.asarray(latent_states)
    latent_actions = np.asarray(latent_actions)
    state_space_samples = np.asarray(state_space_samples)
    action_space_samples = np.asarray(action_space_samples)
    nc = _get_nc()
    in_maps = _make_in_maps(latent_states, latent_actions,
                            state_space_samples, action_space_samples)
    res = run_bass_kernel_spmd(nc, in_maps, core_ids=list(range(8)),
                               trace=_trace)
    out = _host_combine(res.results, latent_states, latent_actions,
                        state_space_samples, action_space_samples)
    if _want_results:
        return out, res
    return out
